# revision 1
# baseline (speedup 1.0000x reference)
"""Causal self-attention Trainium2 kernel (8 NeuronCores).

Sharding: core = b*2 + g where b = batch (4), g = head-group (2 groups x 8 heads).
Each core computes, for its (batch, head-group):
    qkv = x[b] @ w_attn[:, group cols] + b_attn[group]
    y_g = softmax_causal(q k^T / sqrt(hs)) v          (8 heads)
    part = y_g @ w_proj[group rows, :] (+ b_proj on g==0)
Host sums the two per-batch partials (the c_proj row-split reduction).

v2 design (vs the DRAM-round-trip baseline):
  - x transpose via PE (is_transpose matmuls through an identity), fused with
    the f32->bf16 cast on the psum->sbuf copies. No x DRAM round trip, no
    XBAR ordering hazards.
  - Software-pipelined emission: the transposes + qkv matmuls of block n+1
    and the projection of block n-1 are injected between attention heads of
    block n, so the PE stays fed while the scalar engine runs the exp chain.
  - Softmax denominators: ones-row in V accumulates colsums in PSUM; the
    reciprocal rows are broadcast across partitions with tiny PE matmuls
    (ones stationary) instead of a DRAM round trip.
  - exp on scalar (ACT); copies split ACT/DVE; casts DVE/ACT/GpSimd;
    y partials stored bf16 (host accumulates in f32).
"""

import sys

sys.path.insert(0, "/opt/trn_rl_repo")

import math
import numpy as np
import ml_dtypes

import concourse.bass as bass
import concourse.bacc as bacc
import concourse.tile as tile
from concourse import mybir
from concourse import bass_utils
from concourse import masks as masks_mod


def _ensure_ntff_hook():
    """Provide antenv.axon_hooks (NTFF profiling registry) if the image's
    antenv lacks it, wiring the ctypes-based hook from trn_agent_boot."""
    import types
    try:
        import antenv.axon_hooks  # noqa: F401
        return
    except ImportError:
        pass
    try:
        import antenv
        from trn_agent_boot.trn_boot import _ntff_profile_via_ctypes
        hook = _ntff_profile_via_ctypes("/opt/axon/libaxon_pjrt.so")
    except Exception:
        return
    mod = types.ModuleType("antenv.axon_hooks")
    mod.get_axon_ntff_profile_hook = lambda: hook
    mod.set_axon_ntff_profile_hook = lambda h: None
    sys.modules["antenv.axon_hooks"] = mod
    antenv.axon_hooks = mod


_ensure_ntff_hook()

F32 = mybir.dt.float32
BF16 = mybir.dt.bfloat16
AF = mybir.ActivationFunctionType
ALU = mybir.AluOpType

T = 2048
C = 1024
HS = 64           # head size
NHL = 8           # heads per core
GC = NHL * HS     # 512: group width
CK = C // 128     # 8 contraction tiles for qkv
MT = T // 128     # 16 row tiles
QB = 512          # q block (one fp32 PSUM bank)
NQ = T // QB      # 4
SCALE = 1.0 / math.sqrt(HS)
N_CORES = 8


def build_program():
    nc = bacc.Bacc("TRN2", target_bir_lowering=False, debug=False, num_devices=N_CORES)
    x_d = nc.dram_tensor("x", [T, C], F32, kind="ExternalInput").ap()
    wqkv_d = nc.dram_tensor("w_qkv", [C, 3 * GC], BF16, kind="ExternalInput").ap()
    bqk_d = nc.dram_tensor("b_qk", [128, 8], F32, kind="ExternalInput").ap()
    bv_d = nc.dram_tensor("b_v", [GC], F32, kind="ExternalInput").ap()
    wproj_d = nc.dram_tensor("w_proj", [GC, C], BF16, kind="ExternalInput").ap()
    bproj_d = nc.dram_tensor("b_proj", [C], F32, kind="ExternalInput").ap()
    masks_d = nc.dram_tensor("masks", [4, 128, QB], BF16, kind="ExternalInput").ap()
    y_d = nc.dram_tensor("y", [T, C], BF16, kind="ExternalOutput").ap()

    def bcast(ap, parts):
        # replicate a [1, N] slice across `parts` partitions (DMA source AP)
        return bass.AP(tensor=ap.tensor, offset=ap.offset, ap=[[0, parts]] + list(ap.ap)[-1:])

    with tile.TileContext(nc) as tc:
        from contextlib import ExitStack

        with ExitStack() as ctx:
            const = ctx.enter_context(tc.tile_pool(name="const", bufs=1))
            xf = ctx.enter_context(tc.tile_pool(name="xf", bufs=4))
            xbp = ctx.enter_context(tc.tile_pool(name="xbp", bufs=4))
            sexp = ctx.enter_context(tc.tile_pool(name="sexp", bufs=6))
            csp = ctx.enter_context(tc.tile_pool(name="csp", bufs=4))
            nrm = ctx.enter_context(tc.tile_pool(name="nrm", bufs=4))
            ost = ctx.enter_context(tc.tile_pool(name="ost", bufs=3))
            pss = ctx.enter_context(tc.tile_pool(name="pss", bufs=2, space="PSUM"))
            pys = ctx.enter_context(tc.tile_pool(name="pys", bufs=2, space="PSUM"))
            pgen = ctx.enter_context(tc.tile_pool(name="pgen", bufs=2, space="PSUM"))

            # ---------------- constants ----------------
            # Emission order matters per DMA queue: identity first on gpsimd
            # (needed by the first transposes), block-0 x loads next, w_qkv
            # split round-robin across all three queues (the first qkv matmul
            # needs every chunk), late-use constants last on scalar.
            identity = const.tile([128, 128], BF16)
            masks_mod.make_identity(nc, identity)
            ones = const.tile([128, HS], BF16)
            nc.vector.memset(ones, 1.0)

            xf32 = {}
            xb = {}
            xf_q = {0: nc.sync, 1: nc.gpsimd, 2: nc.sync, 3: nc.gpsimd}

            def emit_loads(n):
                for t in range(4 * n, 4 * n + 4):
                    xt = xf.tile([128, C], F32, tag="xf", name=f"xf{t}")
                    xf_q[t % 4].dma_start(out=xt, in_=x_d[t * 128:(t + 1) * 128, :])
                    xf32[t] = xt

            emit_loads(0)
            b_qk = const.tile([128, 8], F32)
            nc.scalar.dma_start(out=b_qk, in_=bqk_d)
            b_v = const.tile([128, GC], F32)
            nc.scalar.dma_start(out=b_v, in_=bcast(bv_d, 128))
            w_qkv = const.tile([128, CK, 3 * GC], BF16)
            wq_r = wqkv_d.rearrange("(c p) n -> p c n", p=128)
            for c in range(CK):
                nc.scalar.dma_start(out=w_qkv[:, c, :], in_=wq_r[:, c, :])
            mask = const.tile([128, 4, QB], BF16)
            nc.scalar.dma_start(out=mask, in_=masks_d.rearrange("d p q -> p d q"))
            w_proj = const.tile([128, 4, C], BF16)
            nc.scalar.dma_start(out=w_proj, in_=wproj_d.rearrange("(c p) n -> p c n", p=128))
            b_proj = const.tile([128, C], F32)
            nc.scalar.dma_start(out=b_proj, in_=bcast(bproj_d, 128))

            # ---------------- persistent activations ----------------
            xT = const.tile([128, CK, T], BF16)
            qkT = const.tile([128, 8, T], BF16)   # m 0..3 = q cols, 4..7 = k cols
            v = const.tile([128, MT, NHL, HS + 1], BF16)
            nc.vector.memset(v[:, :, :, HS:HS + 1], 1.0)
            yTu = const.tile([128, 4, T], BF16)   # unnormalized y^T (head-dim major)

            def make_tq_closures(n, with_load=True):
                """load + cast + PE-transpose + qkv matmuls for token block n.

                The loads MUST be emitted with their block (not upfront): a
                DMA trigger whose xf-ring slot is freed by a cast that sits
                behind the trigger in the same engine stream deadlocks the
                queue.
                """
                cls = []
                if with_load:
                    cls.append(lambda: emit_loads(n))
                for t in range(4 * n, 4 * n + 4):
                    def CAST(t=t):
                        xbt = xbp.tile([128, C], BF16, tag="xb")
                        # block 0 avoids gpsimd: its ~3.7us cast would sit on
                        # the critical path to the first qkv matmuls
                        if t % 4 == 1:
                            nc.scalar.copy(out=xbt, in_=xf32[t])
                        elif t % 4 == 3 and n > 0:
                            nc.gpsimd.tensor_copy(out=xbt, in_=xf32[t])
                        else:
                            nc.vector.tensor_copy(out=xbt, in_=xf32[t])
                        xb[t] = xbt
                    cls.append(CAST)
                for t in range(4 * n, 4 * n + 4):
                    for q in range(2):
                        def TP(t=t, q=q):
                            tp = pgen.tile([128, 4, 128], BF16, tag="mm")
                            for j in range(4):
                                c = 4 * q + j
                                nc.tensor.transpose(
                                    tp[:, j, :], xb[t][:, c * 128:(c + 1) * 128], identity)
                            eng = nc.scalar if (t % 4 == 3) else nc.vector
                            eng_copy = eng.copy if eng is nc.scalar else eng.tensor_copy
                            eng_copy(out=xT[:, 4 * q:4 * q + 4, t * 128:(t + 1) * 128], in_=tp)
                        cls.append(TP)
                cols = slice(n * QB, (n + 1) * QB)
                for m in range(8):
                    def QK(m=m):
                        ps = pgen.tile([128, QB], F32, tag="mm")
                        for c in range(CK):
                            nc.tensor.matmul(ps,
                                             lhsT=w_qkv[:, c, m * 128:(m + 1) * 128],
                                             rhs=xT[:, c, cols],
                                             start=(c == 0), stop=(c == CK - 1))
                        nc.vector.tensor_scalar_add(out=qkT[:, m, cols],
                                                    in0=ps, scalar1=b_qk[:, m:m + 1])
                    cls.append(QK)
                for t in range(4 * n, 4 * n + 4):
                    def VMM(t=t):
                        ps = pgen.tile([128, QB], F32, tag="mm")
                        for c in range(CK):
                            nc.tensor.matmul(ps,
                                             lhsT=xT[:, c, t * 128:(t + 1) * 128],
                                             rhs=w_qkv[:, c, 2 * GC:3 * GC],
                                             start=(c == 0), stop=(c == CK - 1))
                        nc.vector.tensor_tensor(out=v[:, t, :, 0:HS],
                                                in0=ps.rearrange("p (h d) -> p h d", d=HS),
                                                in1=b_v.rearrange("p (h d) -> p h d", d=HS),
                                                op=ALU.add)
                    cls.append(VMM)
                return cls

            def make_proj_closures(n):
                """projection of q-block n (requires normalized yTu block n)."""
                cls = []
                for t in range(4 * n, 4 * n + 4):
                    for n2 in range(2):
                        def P(t=t, n2=n2):
                            ps = pgen.tile([128, QB], F32, tag="mm")
                            for c4 in range(4):
                                nc.tensor.matmul(ps,
                                                 lhsT=yTu[:, c4, t * 128:(t + 1) * 128],
                                                 rhs=w_proj[:, c4, n2 * QB:(n2 + 1) * QB],
                                                 start=(c4 == 0), stop=(c4 == 3))
                            ot = ost.tile([128, QB], BF16, tag="ot")
                            nc.vector.tensor_tensor(out=ot, in0=ps,
                                                    in1=b_proj[:, n2 * QB:(n2 + 1) * QB],
                                                    op=ALU.add)
                            nc.sync.dma_start(
                                out=y_d[t * 128:(t + 1) * 128, n2 * QB:(n2 + 1) * QB],
                                in_=ot)
                        cls.append(P)
                return cls

            cs_tiles = {}

            def emit_head(qj, h, filler):
                """attention for head h on q-block qj; pops one injected
                closure from `filler` after each k-chunk so the PE has work
                while the scalar engine runs the exp chain."""
                cols = slice(qj * QB, (qj + 1) * QB)
                nki = 4 * (qj + 1)
                hp, po = h // 2, (h % 2) * HS
                if h % 2 == 0:
                    cs_tiles[hp] = csp.tile([128, QB], F32, tag="cs",
                                            name=f"cs{qj}_{hp}")
                py = pys.tile([HS + 1, QB], F32, tag="py")
                for kp in range(nki // 2):
                    ps = pss.tile([128, 2, QB], F32, tag="s")
                    for j in range(2):
                        ki = 2 * kp + j
                        nc.tensor.matmul(ps[:, j, :],
                                         lhsT=qkT[po:po + HS, 4 + hp, ki * 128:(ki + 1) * 128],
                                         rhs=qkT[po:po + HS, hp, cols],
                                         start=True, stop=True)
                    ex = sexp.tile([128, 2, QB], BF16, tag="e")
                    nc.scalar.activation(out=ex, in_=ps, func=AF.Exp, scale=SCALE)
                    d = 2 * kp - 4 * qj
                    if d >= 0:
                        nc.vector.tensor_tensor(out=ex, in0=ex,
                                                in1=mask[:, d:d + 2, :],
                                                op=ALU.mult)
                    for j in range(2):
                        ki = 2 * kp + j
                        nc.tensor.matmul(py, lhsT=v[:, ki, h, :],
                                         rhs=ex[:, j, :],
                                         start=(ki == 0), stop=(ki == nki - 1))
                    if filler:
                        filler.pop(0)()
                # yTu copy: DVE when the output starts at partition 0, ACT for
                # the partition-shifted (base 64) case which DVE mishandles.
                if po == 0:
                    nc.vector.tensor_copy(out=yTu[0:HS, hp, cols], in_=py[0:HS, :])
                else:
                    nc.scalar.copy(out=yTu[po:po + HS, hp, cols], in_=py[0:HS, :])
                # head 2hp -> row 0, head 2hp+1 -> row 64 (bases {0,64} only:
                # engine ops and matmul operands can't start at partition 96).
                # The row-64 copy is base-aligned (64->64) so DVE handles it.
                if po == 0:
                    nc.scalar.copy(out=cs_tiles[hp][0:1, :], in_=py[HS:HS + 1, :])
                else:
                    nc.vector.tensor_copy(out=cs_tiles[hp][po:po + 1, :],
                                          in_=py[HS:HS + 1, :])

            def emit_normalize(qj, hp):
                """reciprocal + PE partition-broadcast + normalize for head
                pair hp (heads 2hp, 2hp+1) of q-block qj."""
                cols = slice(qj * QB, (qj + 1) * QB)
                cst = cs_tiles[hp]
                rcp = nrm.tile([128, QB], F32, tag="rcp")
                nc.vector.reciprocal_approx_fast(out=rcp, in_=cst)
                r16 = nrm.tile([128, QB], BF16, tag="r16")
                nc.vector.tensor_copy(out=r16, in_=rcp)
                rb = pgen.tile([128, QB], F32, tag="mm")
                for k in range(2):          # heads within the pair
                    rp = k * HS
                    nc.tensor.matmul(rb[rp:rp + HS, :],
                                     lhsT=ones[rp:rp + 1, :],
                                     rhs=r16[rp:rp + 1, :],
                                     start=True, stop=True)
                nc.vector.tensor_tensor(out=yTu[:, hp, cols],
                                        in0=yTu[:, hp, cols],
                                        in1=rb, op=ALU.mult)

            # ---------------- pipelined emission ----------------
            for cl in make_tq_closures(0, with_load=False):
                cl()
            for qj in range(NQ):
                inj = []
                if qj + 1 < NQ:
                    inj += make_tq_closures(qj + 1)
                if qj >= 1:
                    inj += make_proj_closures(qj - 1)
                for h in range(NHL):
                    # hand this head its even share of the filler closures
                    per = (len(inj) + (NHL - h) - 1) // (NHL - h)
                    filler = inj[:per]
                    del inj[:per]
                    emit_head(qj, h, filler)
                    if h % 2 == 1:
                        emit_normalize(qj, h // 2)
                    for cl in filler:   # anything the chunks didn't absorb
                        cl()
            for cl in make_proj_closures(NQ - 1):
                cl()

    nc.compile()
    return nc


def make_masks():
    kk = np.arange(128)[:, None]
    qq = np.arange(QB)[None, :]
    m = np.zeros((4, 128, QB), dtype=ml_dtypes.bfloat16)
    for d in range(4):
        m[d] = (qq >= kk + d * 128).astype(ml_dtypes.bfloat16)
    return m


def make_in_maps(x, w_attn, b_attn, w_proj, b_proj):
    masks = make_masks()
    in_maps = []
    for core in range(N_CORES):
        b, g = core // 2, core % 2
        cq = slice(g * GC, (g + 1) * GC)
        ck = slice(C + g * GC, C + (g + 1) * GC)
        cv = slice(2 * C + g * GC, 2 * C + (g + 1) * GC)
        w_qkv_g = np.concatenate([w_attn[:, cq], w_attn[:, ck], w_attn[:, cv]], axis=1)
        in_maps.append({
            "x": np.ascontiguousarray(np.asarray(x[b], dtype=np.float32)),
            # pre-tiled [128, 8]: b_qk[p, m] = flat[m*128 + p] (contiguous DMA)
            "w_qkv": np.ascontiguousarray(w_qkv_g.astype(ml_dtypes.bfloat16)),
            "b_qk": np.ascontiguousarray(
                np.concatenate([b_attn[cq], b_attn[ck]]).astype(np.float32)
                .reshape(8, 128).T),
            "b_v": np.ascontiguousarray(b_attn[cv]).astype(np.float32),
            "w_proj": np.ascontiguousarray(w_proj[g * GC:(g + 1) * GC, :].astype(ml_dtypes.bfloat16)),
            "b_proj": (b_proj if g == 0 else np.zeros_like(b_proj)).astype(np.float32),
            "masks": masks,
        })
    return in_maps


_PROGRAM = None


def kernel(x, w_attn, b_attn, w_proj, b_proj, _trace=False):
    global _PROGRAM
    x = np.asarray(x)
    B = x.shape[0]
    if _PROGRAM is None:
        _PROGRAM = build_program()
    nc = _PROGRAM
    in_maps = make_in_maps(x, np.asarray(w_attn), np.asarray(b_attn),
                           np.asarray(w_proj), np.asarray(b_proj))
    res = bass_utils.run_bass_kernel_spmd(nc, in_maps, core_ids=list(range(N_CORES)),
                                          trace=_trace)
    y = np.zeros((B, T, C), np.float32)
    for b in range(B):
        y[b] = (res.results[2 * b]["y"].astype(np.float32)
                + res.results[2 * b + 1]["y"].astype(np.float32))
    if _trace:
        return y, res
    return y



# revision 5
# speedup vs baseline: 1.1038x; 1.1038x over previous
"""Causal self-attention Trainium2 kernel (8 NeuronCores).

Sharding: core = b*2 + g where b = batch (4), g = head-group (2 groups x 8 heads).
Each core computes, for its (batch, head-group):
    qkv = x[b] @ w_attn[:, group cols] + b_attn[group]
    y_g = softmax_causal(q k^T / sqrt(hs)) v          (8 heads)
    part = y_g @ w_proj[group rows, :] (+ b_proj on g==0)
Host sums the two per-batch partials (the c_proj row-split reduction).

v3 design (vs v2):
  - x is transposed + cast to bf16 on the host: xT DMAs straight into its
    [128, CK, T] SBUF layout. Kills the PE-transpose pipeline (128 transposes,
    psum->sbuf copies, f32->bf16 casts) and halves the x DMA bytes.
  - Attention runs per head PAIR: the two heads of a pair live on partition
    halves 0-63 / 64-127 of qkT, so their K=64 QK^T matmuls map to PE row
    tiles (0,0) and (64,0) and stream concurrently (2x QK throughput).
  - Causal diagonal blocks are range-restricted: QK^T per-tile valid-q lower
    bound, exp/mask/AV at pair granularity. Skips ~60% of the wasted
    masked-region work on PE and ACT.
  - w_qkv is re-tiled on host per 128-col m-block so the first QK matmul only
    waits on a 256KB DMA; xT DMAs are per 512-token block.
  - exp on scalar (ACT); bias adds / mask / normalize on DVE; y partials
    stored bf16 (host accumulates in f32).
"""

import sys

sys.path.insert(0, "/opt/trn_rl_repo")

import math
import numpy as np
import ml_dtypes

import concourse.bass as bass
import concourse.bacc as bacc
import concourse.tile as tile
from concourse import mybir
from concourse import bass_utils


def _ensure_ntff_hook():
    """Provide antenv.axon_hooks (NTFF profiling registry) if the image's
    antenv lacks it, wiring the ctypes-based hook from trn_agent_boot."""
    import types
    try:
        import antenv.axon_hooks  # noqa: F401
        return
    except ImportError:
        pass
    try:
        import antenv
        from trn_agent_boot.trn_boot import _ntff_profile_via_ctypes
        hook = _ntff_profile_via_ctypes("/opt/axon/libaxon_pjrt.so")
    except Exception:
        return
    mod = types.ModuleType("antenv.axon_hooks")
    mod.get_axon_ntff_profile_hook = lambda: hook
    mod.set_axon_ntff_profile_hook = lambda h: None
    sys.modules["antenv.axon_hooks"] = mod
    antenv.axon_hooks = mod


_ensure_ntff_hook()

F32 = mybir.dt.float32
BF16 = mybir.dt.bfloat16
AF = mybir.ActivationFunctionType
ALU = mybir.AluOpType

T = 2048
C = 1024
HS = 64           # head size
NHL = 8           # heads per core
GC = NHL * HS     # 512: group width
CK = C // 128     # 8 contraction tiles for qkv
MT = T // 128     # 16 row tiles
QB = 512          # q block (one fp32 PSUM bank)
NQ = T // QB      # 4
NM = 3 * GC // 128  # 12 m-blocks of w_qkv
SCALE = 1.0 / math.sqrt(HS)
N_CORES = 8


def build_program():
    nc = bacc.Bacc("TRN2", target_bir_lowering=False, debug=False, num_devices=N_CORES)
    xT_d = nc.dram_tensor("xT", [C, T], BF16, kind="ExternalInput").ap()
    wqkv_d = nc.dram_tensor("w_qkv", [NM, 128, CK * 128], BF16, kind="ExternalInput").ap()
    bqk_d = nc.dram_tensor("b_qk", [128, 8], F32, kind="ExternalInput").ap()
    bv_d = nc.dram_tensor("b_v", [GC], F32, kind="ExternalInput").ap()
    wproj_d = nc.dram_tensor("w_proj", [GC, C], BF16, kind="ExternalInput").ap()
    bproj_d = nc.dram_tensor("b_proj", [C], F32, kind="ExternalInput").ap()
    masks_d = nc.dram_tensor("masks", [4, 128, QB], BF16, kind="ExternalInput").ap()
    y_d = nc.dram_tensor("y", [T, C], BF16, kind="ExternalOutput").ap()

    def bcast(ap, parts):
        # replicate a [1, N] slice across `parts` partitions (DMA source AP)
        return bass.AP(tensor=ap.tensor, offset=ap.offset, ap=[[0, parts]] + list(ap.ap)[-1:])

    with tile.TileContext(nc) as tc:
        from contextlib import ExitStack

        with ExitStack() as ctx:
            const = ctx.enter_context(tc.tile_pool(name="const", bufs=1))
            sexp = ctx.enter_context(tc.tile_pool(name="sexp", bufs=6))
            csp = ctx.enter_context(tc.tile_pool(name="csp", bufs=4))
            nrm = ctx.enter_context(tc.tile_pool(name="nrm", bufs=4))
            ost = ctx.enter_context(tc.tile_pool(name="ost", bufs=3))
            pss = ctx.enter_context(tc.tile_pool(name="pss", bufs=2, space="PSUM"))
            pys = ctx.enter_context(tc.tile_pool(name="pys", bufs=2, space="PSUM"))
            pgen = ctx.enter_context(tc.tile_pool(name="pgen", bufs=2, space="PSUM"))

            # ---------------- persistent activations ----------------
            xT = const.tile([128, CK, T], BF16)
            qkT = const.tile([128, 8, T], BF16)   # m 0..3 = q cols, 4..7 = k cols
            v = const.tile([128, MT, NHL, HS + 1], BF16)
            yTu = const.tile([128, 4, T], BF16)   # unnormalized y^T (head-dim major)

            # ---------------- constants + input DMAs ----------------
            # The first QK matmul (block 0, m=0) needs w_qkv m-block 0 and xT
            # block 0 only; order the DMAs so those land first, split across
            # the sync (w) and gpsimd (xT) queues.
            w_qkv = const.tile([128, CK, NM * 128], BF16)
            ones = const.tile([128, HS], BF16)
            nc.vector.memset(ones, 1.0)
            nc.vector.memset(v[:, :, :, HS:HS + 1], 1.0)

            wq_src = [wqkv_d[j].rearrange("p (c n) -> p c n", c=CK) for j in range(NM)]
            nc.sync.dma_start(out=w_qkv[:, :, 0:128], in_=wq_src[0])
            xT_r = xT_d.rearrange("(c p) t -> p c t", p=128)
            nc.gpsimd.dma_start(out=xT[:, :, 0:QB], in_=xT_r[:, :, 0:QB])
            for j in range(1, NM):
                nc.sync.dma_start(out=w_qkv[:, :, j * 128:(j + 1) * 128], in_=wq_src[j])
            for blk in range(1, NQ):
                nc.gpsimd.dma_start(out=xT[:, :, blk * QB:(blk + 1) * QB],
                                    in_=xT_r[:, :, blk * QB:(blk + 1) * QB])
            b_qk = const.tile([128, 8], F32)
            nc.gpsimd.dma_start(out=b_qk, in_=bqk_d)
            b_v = const.tile([128, GC], F32)
            nc.gpsimd.dma_start(out=b_v, in_=bcast(bv_d, 128))
            mask = const.tile([128, 4, QB], BF16)
            nc.gpsimd.dma_start(out=mask, in_=masks_d.rearrange("d p q -> p d q"))
            w_proj = const.tile([128, 4, C], BF16)
            nc.gpsimd.dma_start(out=w_proj, in_=wproj_d.rearrange("(c p) n -> p c n", p=128))
            b_proj = const.tile([128, C], F32)
            nc.gpsimd.dma_start(out=b_proj, in_=bcast(bproj_d, 128))

            def make_tq_closures(n):
                """qkv matmuls for token block n (xT already streaming in)."""
                cls = []
                cols = slice(n * QB, (n + 1) * QB)
                for m in range(8):
                    def QK(m=m):
                        ps = pgen.tile([128, QB], F32, tag="mm")
                        for c in range(CK):
                            nc.tensor.matmul(ps,
                                             lhsT=w_qkv[:, c, m * 128:(m + 1) * 128],
                                             rhs=xT[:, c, cols],
                                             start=(c == 0), stop=(c == CK - 1))
                        nc.vector.tensor_scalar_add(out=qkT[:, m, cols],
                                                    in0=ps, scalar1=b_qk[:, m:m + 1])
                    cls.append(QK)
                for t in range(4 * n, 4 * n + 4):
                    def VMM(t=t):
                        ps = pgen.tile([128, QB], F32, tag="mm")
                        for c in range(CK):
                            nc.tensor.matmul(ps,
                                             lhsT=xT[:, c, t * 128:(t + 1) * 128],
                                             rhs=w_qkv[:, c, 8 * 128:],
                                             start=(c == 0), stop=(c == CK - 1))
                        nc.vector.tensor_tensor(out=v[:, t, :, 0:HS],
                                                in0=ps.rearrange("p (h d) -> p h d", d=HS),
                                                in1=b_v.rearrange("p (h d) -> p h d", d=HS),
                                                op=ALU.add)
                    cls.append(VMM)
                return cls

            def make_proj_closures(n):
                """projection of q-block n (requires normalized yTu block n)."""
                cls = []
                for t in range(4 * n, 4 * n + 4):
                    for n2 in range(2):
                        def P(t=t, n2=n2):
                            ps = pgen.tile([128, QB], F32, tag="mm")
                            for c4 in range(4):
                                nc.tensor.matmul(ps,
                                                 lhsT=yTu[:, c4, t * 128:(t + 1) * 128],
                                                 rhs=w_proj[:, c4, n2 * QB:(n2 + 1) * QB],
                                                 start=(c4 == 0), stop=(c4 == 3))
                            ot = ost.tile([128, QB], BF16, tag="ot")
                            nc.vector.tensor_tensor(out=ot, in0=ps,
                                                    in1=b_proj[:, n2 * QB:(n2 + 1) * QB],
                                                    op=ALU.add)
                            nc.sync.dma_start(
                                out=y_d[t * 128:(t + 1) * 128, n2 * QB:(n2 + 1) * QB],
                                in_=ot)
                        cls.append(P)
                return cls

            def emit_pair(qj, hp, filler):
                """attention for head pair (2hp, 2hp+1) on q-block qj.

                Head A = 2hp lives on partitions 0-63, head B = 2hp+1 on
                64-127 of qkT column hp (q) / 4+hp (k): the pair's QK^T
                matmuls are emitted back-to-back so they run concurrently on
                PE row tiles (0,0) and (64,0).  AV(kp-1) trails QK(kp) by one
                step; one filler closure (qkv of block qj+1 / proj of qj-1)
                is popped per step to keep the PE fed under the exp chain.
                """
                q0 = qj * QB
                nki = 4 * (qj + 1)
                exs = {}
                plos = {}
                pyA = pys.tile([HS + 1, QB], F32, tag="py", name=f"pyA{qj}_{hp}")
                pyB = pys.tile([HS + 1, QB], F32, tag="py", name=f"pyB{qj}_{hp}")
                for kp in range(nki // 2 + 1):
                    if kp < nki // 2:
                        # valid-q lower bound per ki (diagonal restriction)
                        los = [max(0, (2 * kp + j - 4 * qj) * 128) for j in range(2)]
                        plo = los[0]
                        psA = pss.tile([128, 2, QB], F32, tag="s")
                        psB = pss.tile([128, 2, QB], F32, tag="s")
                        for j in range(2):
                            ki = 2 * kp + j
                            lo = los[j]
                            nc.tensor.matmul(psA[:, j, lo:],
                                             lhsT=qkT[0:HS, 4 + hp, ki * 128:(ki + 1) * 128],
                                             rhs=qkT[0:HS, hp, q0 + lo:q0 + QB],
                                             start=True, stop=True)
                            nc.tensor.matmul(psB[:, j, lo:],
                                             lhsT=qkT[HS:128, 4 + hp, ki * 128:(ki + 1) * 128],
                                             rhs=qkT[HS:128, hp, q0 + lo:q0 + QB],
                                             start=True, stop=True)
                        exA = sexp.tile([128, 2, QB], BF16, tag="e")
                        exB = sexp.tile([128, 2, QB], BF16, tag="e")
                        nc.scalar.activation(out=exA[:, :, plo:], in_=psA[:, :, plo:],
                                             func=AF.Exp, scale=SCALE)
                        nc.scalar.activation(out=exB[:, :, plo:], in_=psB[:, :, plo:],
                                             func=AF.Exp, scale=SCALE)
                        d = 2 * kp - 4 * qj
                        if d >= 0:
                            # zeroes the masked region (incl. garbage cols of
                            # the restricted j=1 tile between plo and its lo)
                            nc.vector.tensor_tensor(out=exA[:, :, plo:], in0=exA[:, :, plo:],
                                                    in1=mask[:, d:d + 2, plo:],
                                                    op=ALU.mult)
                            nc.vector.tensor_tensor(out=exB[:, :, plo:], in0=exB[:, :, plo:],
                                                    in1=mask[:, d:d + 2, plo:],
                                                    op=ALU.mult)
                        exs[kp] = (exA, exB)
                        plos[kp] = plo
                    if kp > 0:
                        exA, exB = exs.pop(kp - 1)
                        plo = plos[kp - 1]
                        for j in range(2):
                            ki = 2 * (kp - 1) + j
                            nc.tensor.matmul(pyA[:, plo:], lhsT=v[:, ki, 2 * hp, :],
                                             rhs=exA[:, j, plo:],
                                             start=(ki == 0), stop=(ki == nki - 1))
                            nc.tensor.matmul(pyB[:, plo:], lhsT=v[:, ki, 2 * hp + 1, :],
                                             rhs=exB[:, j, plo:],
                                             start=(ki == 0), stop=(ki == nki - 1))
                        steps_left = nki // 2 - (kp - 1)
                        n_pop = (len(filler) + steps_left - 1) // steps_left
                        for _ in range(n_pop):
                            filler.pop(0)()
                return pyA, pyB

            def emit_finish_pair(qj, hp, pyA, pyB):
                """denominators + copies + normalize for head pair hp."""
                cols = slice(qj * QB, (qj + 1) * QB)
                cst = csp.tile([128, QB], F32, tag="cs")
                # denominator rows -> partitions 0 / 64 of one tile
                nc.scalar.copy(out=cst[0:1, :], in_=pyA[HS:HS + 1, :])
                nc.vector.tensor_copy(out=cst[HS:HS + 1, :], in_=pyB[HS:HS + 1, :])
                # y^T copies: A base-aligned on DVE, B partition-shifted on ACT
                nc.vector.tensor_copy(out=yTu[0:HS, hp, cols], in_=pyA[0:HS, :])
                nc.scalar.copy(out=yTu[HS:128, hp, cols], in_=pyB[0:HS, :])
                rcp = nrm.tile([128, QB], F32, tag="rcp")
                nc.vector.reciprocal_approx_fast(out=rcp, in_=cst)
                r16 = nrm.tile([128, QB], BF16, tag="r16")
                nc.vector.tensor_copy(out=r16, in_=rcp)
                rb = pgen.tile([128, QB], F32, tag="mm")
                for k in range(2):          # heads within the pair
                    rp = k * HS
                    nc.tensor.matmul(rb[rp:rp + HS, :],
                                     lhsT=ones[rp:rp + 1, :],
                                     rhs=r16[rp:rp + 1, :],
                                     start=True, stop=True)
                nc.vector.tensor_tensor(out=yTu[:, hp, cols],
                                        in0=yTu[:, hp, cols],
                                        in1=rb, op=ALU.mult)

            # ---------------- pipelined emission ----------------
            for cl in make_tq_closures(0):
                cl()
            for qj in range(NQ):
                inj = []
                if qj + 1 < NQ:
                    inj += make_tq_closures(qj + 1)
                if qj >= 1:
                    inj += make_proj_closures(qj - 1)
                for hp in range(4):
                    # hand this pair its even share of the filler closures
                    per = (len(inj) + (4 - hp) - 1) // (4 - hp)
                    filler = inj[:per]
                    del inj[:per]
                    pyA, pyB = emit_pair(qj, hp, filler)
                    emit_finish_pair(qj, hp, pyA, pyB)
            for cl in make_proj_closures(NQ - 1):
                cl()

    nc.compile()
    return nc


def make_masks():
    kk = np.arange(128)[:, None]
    qq = np.arange(QB)[None, :]
    m = np.zeros((4, 128, QB), dtype=ml_dtypes.bfloat16)
    for d in range(4):
        m[d] = (qq >= kk + d * 128).astype(ml_dtypes.bfloat16)
    return m


def make_in_maps(x, w_attn, b_attn, w_proj, b_proj):
    masks = make_masks()
    in_maps = []
    for core in range(N_CORES):
        b, g = core // 2, core % 2
        cq = slice(g * GC, (g + 1) * GC)
        ck = slice(C + g * GC, C + (g + 1) * GC)
        cv = slice(2 * C + g * GC, 2 * C + (g + 1) * GC)
        w_qkv_g = np.concatenate([w_attn[:, cq], w_attn[:, ck], w_attn[:, cv]], axis=1)
        # re-tile per 128-col m-block: [NM, 128 partitions, CK*128] so each
        # m-block DMA reads contiguous 2KB per partition
        w_tiled = np.ascontiguousarray(
            w_qkv_g.astype(ml_dtypes.bfloat16)
            .reshape(CK, 128, NM, 128).transpose(2, 1, 0, 3).reshape(NM, 128, CK * 128))
        in_maps.append({
            "xT": np.ascontiguousarray(np.asarray(x[b], dtype=np.float32).T
                                       .astype(ml_dtypes.bfloat16)),
            "w_qkv": w_tiled,
            # pre-tiled [128, 8]: b_qk[p, m] = flat[m*128 + p] (contiguous DMA)
            "b_qk": np.ascontiguousarray(
                np.concatenate([b_attn[cq], b_attn[ck]]).astype(np.float32)
                .reshape(8, 128).T),
            "b_v": np.ascontiguousarray(b_attn[cv]).astype(np.float32),
            "w_proj": np.ascontiguousarray(w_proj[g * GC:(g + 1) * GC, :].astype(ml_dtypes.bfloat16)),
            "b_proj": (b_proj if g == 0 else np.zeros_like(b_proj)).astype(np.float32),
            "masks": masks,
        })
    return in_maps


_PROGRAM = None


def kernel(x, w_attn, b_attn, w_proj, b_proj, _trace=False):
    global _PROGRAM
    x = np.asarray(x)
    B = x.shape[0]
    if _PROGRAM is None:
        _PROGRAM = build_program()
    nc = _PROGRAM
    in_maps = make_in_maps(x, np.asarray(w_attn), np.asarray(b_attn),
                           np.asarray(w_proj), np.asarray(b_proj))
    res = bass_utils.run_bass_kernel_spmd(nc, in_maps, core_ids=list(range(N_CORES)),
                                          trace=_trace)
    y = np.zeros((B, T, C), np.float32)
    for b in range(B):
        y[b] = (res.results[2 * b]["y"].astype(np.float32)
                + res.results[2 * b + 1]["y"].astype(np.float32))
    if _trace:
        return y, res
    return y
